# revision 80
# baseline (speedup 1.0000x reference)
"""Trainium2 Bass kernel for nn_MultiHeadAttention (B=4, S=2048, D=1024, H=16).

Sharding: 8 cores = batch(4) x head-half(2).  Each core computes, for its
batch element, 8 of the 16 heads: QKV projections against column-sliced
weights, causal attention, and the output projection against the matching
row-slice of Wo.  The two partial outputs per batch element are summed on
the host (replaces the tensor-parallel all-reduce), and Wo_b is added there.

Attention runs in the transposed-scores layout scoresT[k, q]; the softmax
denominator comes free from an all-ones column appended to V (row 64 of the
PV psum accumulator).  Heads are processed in PAIRS sharing one [128, 1024]
scores psum tile (head A in cols 0:512, head B in 512:1024) so one ACTIVATE
exps both heads' scores; q is chunked at 512.

The whole kernel is software-pipelined around the PE matmul stream: Q/K
projections for the NEXT head pair and output-projection tiles for
finished pairs are injected as fill between attention steps.  Fill is
spread over EVERY pass (one q + one k projection chunk per pass, outproj
late-filled after the normalize it needs) because attention-only phases
(64-row scores, 65-col PV) sit at ~50% PE-array activity, which drags the
HAM activity monitor below threshold and re-throttles the PE from 2.4 to
1.2 GHz for whole passes; warm-up matmuls (a contiguous burst at start,
filler chunks in thin-fill passes and across the epilogue normalize)
serve the same purpose where no real fill exists.  The softmax normalize
runs entirely off the PE (gpsimd partition_broadcast of 1/den + DVE
multiplies), deferred one pass; the PV psum accumulator is evicted
incrementally (each 128-column block right after its diagonal step).
V-bias and the causal mask are folded out of the device code (host adds
Wv_b@Wo_w to the output; the mask ships as an input constant).
DMA: only sync/scalar/gpsimd can push descriptors (~650ns each, starting
after the ~6.5us boot barrier), so x arrives host-pre-tiled (contiguous
128KB+ tiles DMA ~2x faster than 4KB-strided rows), descriptors are
few/large and ordered by consumption, and output stores are contiguous
full-row 512KB slices rotated across the three queues.
PSUM: 2 scores bufs (4 banks) + 1 shared PV accumulator (2 banks) + 2
fill bufs (2 banks).
"""

import sys

if "/opt/trn_rl_repo" not in sys.path:
    sys.path.insert(0, "/opt/trn_rl_repo")

import numpy as np
import ml_dtypes

B, S, D = 4, 2048, 1024
H, HD = 16, 64
HH = H // 2          # heads per core
DH = D // 2          # local attention feature dim (HH * HD)
N_CORES = 8
QC = 512             # q-chunk per attention pass (1 psum bank per head)

# matmul dtype mode: "bf16" (fast, ~3e-3 rel err) | "f32" (exact, 4x PE cost)
DT_MODE = "bf16"

_CACHE = {}


def _build(dt_mode):
    import concourse.bass as bass
    import concourse.mybir as mybir
    from concourse import bacc
    from concourse.tile import TileContext

    F32 = mybir.dt.float32
    if dt_mode == "bf16":
        DT = mybir.dt.bfloat16
    elif dt_mode == "f32":
        DT = mybir.dt.float32
    else:
        raise ValueError(dt_mode)
    FP8 = mybir.dt.float8e4
    DR = mybir.MatmulPerfMode.DoubleRow

    ADD = mybir.AluOpType.add
    MULT = mybir.AluOpType.mult
    EXP = mybir.ActivationFunctionType.Exp
    COPY = mybir.ActivationFunctionType.Copy

    nc = bacc.Bacc("TRN2", target_bir_lowering=False, debug=False,
                   num_devices=N_CORES)

    ND = D // 128        # 8 contraction tiles over D
    NS = S // 128        # 16 s-blocks
    NJ = DH // 128       # 4 head-pair tiles of the local 512 dim
    NSC = S // 512       # 4 columns of 512 over S
    NP = S // QC         # 4 q-chunk passes

    # x arrives pre-tiled on the host (strided DMAs with 1KB lines / 4KB
    # stride run at ~85GB/s per queue vs ~170GB/s contiguous).  Per db
    # block of 512 dram rows: sc0 tile [128,512], sc1 tile [128,512],
    # then the sc2+sc3 halves as ONE [128,1024] tile (flattened to
    # [256,512]) -- each dma_start costs ~650ns on its sequencer and the
    # pushes only begin after the ~6.5us startup barrier, so fewer,
    # larger descriptors shorten the DMA ramp.
    xT = nc.dram_tensor("xT", [ND * NSC * 128, 512], DT,
                        kind="ExternalInput").ap()
    wq = nc.dram_tensor("wq", [D, DH], DT, kind="ExternalInput").ap()
    wk = nc.dram_tensor("wk", [D, DH], DT, kind="ExternalInput").ap()
    wv = nc.dram_tensor("wv", [D, DH], DT, kind="ExternalInput").ap()
    wo = nc.dram_tensor("wo", [DH, D], DT, kind="ExternalInput").ap()
    bq = nc.dram_tensor("bq", [128, DH // 128], F32, kind="ExternalInput").ap()
    bk = nc.dram_tensor("bk", [128, DH // 128], F32, kind="ExternalInput").ap()
    # causal mask for diagonal 128x128 squares, two side-by-side copies
    # (one per head of a pair), built on the host
    mask2d = nc.dram_tensor("mask2", [128, 256], DT, kind="ExternalInput").ap()
    # output rows are full 4KB lines -> a [128, D] slice is a contiguous
    # 512KB store; one DMA per s-block
    out = nc.dram_tensor("out", [S, D], F32, kind="ExternalOutput").ap()

    from contextlib import ExitStack

    with TileContext(nc) as tc, ExitStack() as _st:
            def pool(**kw):
                return _st.enter_context(tc.tile_pool(**kw))
            pp = pool(name="persist", bufs=1)
            pxt = pool(name="xt", bufs=ND * 2)
            pxt23 = pool(name="xt23", bufs=ND)
            pwqk = pool(name="wqk", bufs=2 * ND)
            pwv = pool(name="wv", bufs=ND)
            pwo = pool(name="wo", bufs=NJ)
            pqT = pool(name="qT", bufs=NJ)
            pkT = pool(name="kT", bufs=NJ)
            pv = pool(name="vaug", bufs=NS)
            pattnT = pool(name="attnT", bufs=NJ)
            pexp = pool(name="exp", bufs=3)
            pau = pool(name="au", bufs=2)
            pdn = pool(name="dn", bufs=4)
            pbc = pool(name="bc", bufs=2)
            post = pool(name="ostage", bufs=4)
            pscps = pool(name="scps", bufs=2, space="PSUM")
            patps = pool(name="atps", bufs=1, space="PSUM")
            pauxps = pool(name="auxps", bufs=2, space="PSUM")
            # ---- input DMAs (3 HWDGE queues, ordered by first use) ----
            # Only sync (SP), scalar (Activation) and gpsimd can initiate
            # DMAs.  sync: x sc0+sc1 (first Q/K/V operands); scalar:
            # biases + wq then x sc2; gpsimd: wk + wv then x sc3 + wo.
            # Per-queue bandwidth is ~100GB/s; the first projection
            # chunk's operands (wq0/wq1 + xt[0..1][0], ~0.5MB) land
            # ~2.5us in with all three queues pulling concurrently.
            bq_t = pp.tile([128, NJ], F32, tag="bq")
            bk_t = pp.tile([128, NJ], F32, tag="bk")
            mask2 = pp.tile([128, 256], DT, tag="mask2")
            xt_c = [[pxt.tile([128, 512], DT, tag="xt",
                              name=f"xt{db}_{sc}") for sc in range(2)]
                    for db in range(ND)]
            xt23 = [pxt23.tile([128, 1024], DT, tag="xt23",
                               name=f"xt23_{db}") for db in range(ND)]

            def xchunk(db, sc):
                """rhs view of x chunk (db, sc) [128, 512]."""
                if sc < 2:
                    return xt_c[db][sc][:]
                return xt23[db][:, (sc - 2) * 512:(sc - 1) * 512]
            wq_t, wk_t, wv_t = [], [], []
            for db in range(ND):
                tq = pwqk.tile([128, DH], DT, tag="wqk", name=f"wq{db}")
                nc.scalar.dma_start(tq[:], wq[db * 128:(db + 1) * 128, :])
                wq_t.append(tq)
                tk = pwqk.tile([128, DH], DT, tag="wqk", name=f"wk{db}")
                nc.gpsimd.dma_start(tk[:], wk[db * 128:(db + 1) * 128, :])
                wk_t.append(tk)
                nc.sync.dma_start(xt_c[db][0][:],
                                  xT[db * 512:db * 512 + 128, :])
            # biases/mask after wq: their first use (bias-add, diagonal
            # mask) trails the first projection chunk by several us
            nc.scalar.dma_start(bq_t[:], bq[:])
            nc.scalar.dma_start(bk_t[:], bk[:])
            nc.scalar.dma_start(mask2[:], mask2d[:])
            # wv split over scalar+gpsimd so V(0)'s operands land ~13us
            # (it's the third prologue chunk); x sc1 follows sc0 on sync.
            for db in range(ND):
                tv = pwv.tile([128, DH], DT, tag="wv", name=f"wv{db}")
                eng = nc.scalar if db < 4 else nc.gpsimd
                eng.dma_start(tv[:], wv[db * 128:(db + 1) * 128, :])
                wv_t.append(tv)
                nc.sync.dma_start(xt_c[db][1][:],
                                  xT[db * 512 + 128:db * 512 + 256, :])
            # Pre-load the ACT exp table set AFTER all scalar descriptor
            # pushes (the ~2.7us ACT_TABLE_LOAD blocks the scalar
            # sequencer); doing it during the DMA-starved window keeps it
            # off pass 0's critical exp->PV chain.
            pre_exp = pp.tile([1, 16], DT, tag="preexp")
            nc.vector.memset(pre_exp[:], 0.0)
            pre_exp_o = pp.tile([1, 16], DT, tag="preexpo")
            nc.scalar.activation(pre_exp_o[:], pre_exp[:], EXP, scale=1.0)
            for db in range(ND):
                nc.sync.dma_start(
                    xt23[db][:],
                    xT[db * 512 + 256:db * 512 + 512, :].rearrange(
                        "(p a) c -> p (a c)", a=2))
            wo_t = []
            for db in range(NJ):
                t = pwo.tile([128, D], DT, tag="wo", name=f"wo{db}")
                nc.gpsimd.dma_start(t[:], wo[db * 128:(db + 1) * 128, :])
                wo_t.append(t)
            # pre-load gpsimd's broadcast library (used by the softmax
            # normalizes) AFTER its dma_start descriptor pushes: a library
            # load blocks the gpsimd sequencer for several us, so it must
            # not delay the descriptor pushes above, and doing it now (PE
            # still DMA-starved) makes the first real normalize free.
            preb_src = pp.tile([1, 64], F32, tag="prebs")
            nc.vector.memset(preb_src[:], 1.0)
            preb_dst = pp.tile([64, 64], F32, tag="prebd")
            nc.gpsimd.partition_broadcast(preb_dst[:], preb_src[:])

            # ---- constants ----
            ones_t = pp.tile([128, HH], F32, tag="ones")
            nc.vector.memset(ones_t[:], 1.0)
            mask23 = mask2[:].rearrange("p (h c) -> p h c", h=2)
            # warm-up operand: dummy matmuls on this tile keep the PE's
            # HAM activity monitor busy across DMA-starved or
            # dependency-starved stretches (prologue, thin-fill passes,
            # epilogue normalize chain) so the PE clock stays at 2.4GHz.
            warm_t = pp.tile([128, 512], DT, tag="warm")
            nc.vector.memset(warm_t[:], 0.03125)

            # persistent activation buffers
            qT_t = [pqT.tile([128, S], DT, tag="qT", name=f"qT{i}")
                    for i in range(NJ)]
            kT_t = [pkT.tile([128, S], DT, tag="kT", name=f"kT{i}")
                    for i in range(NJ)]
            v_t = [pv.tile([128, HH * (HD + 1)], DT, tag="vaug",
                           name=f"vaug{i}") for i in range(NS)]
            aT_t = [pattnT.tile([128, S], DT, tag="attnT", name=f"attnT{i}")
                    for i in range(NJ)]

            # ---------- fill-work generators (2 matmuls per piece) ----------
            def qk_pieces(j):
                """Q/K projection for head pair j: chunks of ~0.4us pieces.
                A chunk = one psum accumulation group (must not be split
                around another aux-pool allocation)."""
                chunks = []
                for nm, w_t, bias_t, dstT in (
                    ("q", wq_t, bq_t, qT_t), ("k", wk_t, bk_t, kT_t)
                ):
                    for sc in range(NSC):
                        box = {}
                        pieces = []
                        for db0 in range(0, ND, 2):
                            def piece(db0=db0, nm=nm, w_t=w_t, bias_t=bias_t,
                                      dstT=dstT, sc=sc, j=j, box=box):
                                if db0 == 0:
                                    box["t"] = pauxps.tile(
                                        [128, 512], F32, tag="aux",
                                        name=f"qk{nm}{j}_{sc}")
                                for db in (db0, db0 + 1):
                                    nc.tensor.matmul(
                                        box["t"][:],
                                        lhsT=w_t[db][:, j * 128:(j + 1) * 128],
                                        rhs=xchunk(db, sc),
                                        start=(db == 0), stop=(db == ND - 1),
                                    )
                                if db0 == ND - 2:
                                    nc.vector.tensor_scalar_add(
                                        dstT[j][:, sc * 512:(sc + 1) * 512],
                                        box["t"][:], bias_t[:, j:j + 1],
                                    )
                            pieces.append(piece)
                        chunks.append(pieces)
                return chunks

            def v_pieces(sb):
                """V projection for s-block sb (one chunk of 4 pieces)."""
                pieces = []
                box = {}
                for db0 in range(0, ND, 2):
                    def piece(db0=db0, sb=sb, box=box):
                        if db0 == 0:
                            box["t"] = pauxps.tile([128, 512], F32, tag="aux",
                                                   name=f"vps{sb}")
                        for db in (db0, db0 + 1):
                            nc.tensor.matmul(
                                box["t"][:],
                                lhsT=xchunk(db, sb // 4)[:, (sb % 4) * 128:(sb % 4 + 1) * 128],
                                rhs=wv_t[db][:],
                                start=(db == 0), stop=(db == ND - 1),
                            )
                        if db0 == ND - 2:
                            # V bias is NOT added here: (attn+bv)@Wo =
                            # attn@Wo + bv@Wo, and bv@Wo is a constant row
                            # added on the host after the gather.
                            vt = v_t[sb]
                            v3 = vt[:].rearrange("p (h e) -> p h e", e=HD + 1)
                            nc.vector.tensor_copy(
                                v3[:, :, 0:HD],
                                box["t"][:].rearrange("p (h e) -> p h e", e=HD),
                            )
                            nc.vector.tensor_copy(
                                v3[:, :, HD:HD + 1],
                                ones_t[:].rearrange("p (h e) -> p h e", e=1),
                            )
                    pieces.append(piece)
                return [pieces]

            def outproj_pieces(sb):
                """Output projection for s-block sb: 2 chunks of 2 pieces.
                Both jc halves stage into ONE [128, D] tile; a single
                contiguous 512KB row-slice DMA stores the s-block (the
                tail s-blocks split in half across two queues)."""
                chunks = []
                obox = {}
                for jc in range(D // 512):
                    box = {}
                    pieces = []
                    for db0 in range(0, NJ, 2):
                        def piece(db0=db0, sb=sb, jc=jc, box=box):
                            if db0 == 0:
                                box["t"] = pauxps.tile(
                                    [128, 512], F32, tag="aux",
                                    name=f"ops{sb}_{jc}")
                            for db in (db0, db0 + 1):
                                nc.tensor.matmul(
                                    box["t"][:],
                                    lhsT=aT_t[db][:, sb * 128:(sb + 1) * 128],
                                    rhs=wo_t[db][:, jc * 512:(jc + 1) * 512],
                                    start=(db == 0), stop=(db == NJ - 1),
                                )
                            if db0 == NJ - 2:
                                if jc == 0:
                                    obox["t"] = post.tile(
                                        [128, D], F32, tag="ostage",
                                        name=f"ot{sb}")
                                ot = obox["t"]
                                nc.vector.tensor_copy(
                                    ot[:, jc * 512:(jc + 1) * 512],
                                    box["t"][:])
                                if jc == 0:
                                    return
                                qs = (nc.sync, nc.scalar, nc.gpsimd)
                                r0 = sb * 128
                                if sb >= NS - 4:
                                    for hv in range(2):
                                        qs[(sb + hv) % 3].dma_start(
                                            out[r0 + hv * 64:
                                                r0 + (hv + 1) * 64, :],
                                            ot[hv * 64:(hv + 1) * 64, :],
                                        )
                                else:
                                    qs[sb % 3].dma_start(
                                        out[r0:r0 + 128, :], ot[:])
                        pieces.append(piece)
                    chunks.append(pieces)
                return chunks

            # ---------- warm-up matmuls (HAM keep-alive) ----------
            warm_n = [0]

            def warm_mms(n, pool):
                """n dummy matmuls into a fresh tile from `pool`.  pscps is
                safe outside the scores pipeline (prologue/epilogue);
                pauxps-based chunks go through the fill machinery."""
                warm_n[0] += 1
                wp = pool.tile([128, 2 * QC] if pool is pscps else [128, 512],
                               F32, tag="sc" if pool is pscps else "aux",
                               name=f"warm{warm_n[0]}")
                for i in range(n):
                    nc.tensor.matmul(wp[:, 0:512], lhsT=warm_t[:, 0:128],
                                     rhs=warm_t[:],
                                     start=(i == 0), stop=(i == n - 1))

            def warm_chunk(n=4):
                return [lambda: warm_mms(n, pauxps)]

            # ---------- attention for one head pair, with fill ----------
            # pending_norm holds the deferred tail of the previous pass's
            # softmax-normalize: gpsimd partition_broadcast of 1/den + DVE
            # multiplies -- no PE instruction in the chain.  Deferring it
            # one pass keeps the DVE's strict-FIFO queue from head-of-line
            # blocking the exp->mask->PV chain at pass boundaries.
            pending_norm = []

            def attention_pair(j, fill, forced=None, late=None,
                               final=False):
                """fill: per-pass CHUNK-lists (len NP); forced: optional
                {(p, kb): [chunks]} issued right after exp_pv(kb) of pass
                p (hard program-order deadlines for pass-0 operands);
                late: optional {p: [chunks]} appended to pass p's fill
                right after the kb==2 pending-norm pop (for fill that
                reads aT written by that normalize)."""
                forced = forced or {}
                late = late or {}
                vcA = (2 * j) * (HD + 1)
                vcB = (2 * j + 1) * (HD + 1)
                pend_next = None
                for p in range(NP):
                    q0 = p * QC
                    nkb = (q0 + QC) // 128
                    at2 = patps.tile([65, 2 * QC], F32, tag="at",
                                     name=f"at{j}_{p}")
                    at3 = at2[0:65, :].rearrange("p (hh q) -> p hh q", hh=2)
                    final_pass = final and p == NP - 1
                    # au is evicted INCREMENTALLY: 128-q-column block c of
                    # the PV accumulator is final right after diagonal
                    # step kb = 4p+c, so each block is copied out in-loop
                    # and the at2 psum slot frees ~0.3us after the last
                    # PV instead of a 1.2us monolithic copy -- that copy
                    # gated the next pass's first PV (patps has 1 buf).
                    au = None
                    if not final_pass:
                        au = pau.tile([65, 2 * QC], F32, tag="au",
                                      name=f"au{j}_{p}")
                        au3 = au[:].rearrange("p (hh q) -> p hh q", hh=2)
                    # flatten chunks; record the piece indices that are
                    # chunk boundaries (safe points for aux-psum reuse).
                    pfill = [pc for ch in fill[p] for pc in ch]
                    bounds = set()
                    n = 0
                    for ch in fill[p]:
                        bounds.add(n)
                        n += len(ch)
                    bounds.add(n)
                    fi = 0

                    def scores(kb, qq=None, pn=None):
                        qq = q0 if qq is None else qq
                        pn = p if pn is None else pn
                        k0 = kb * 128
                        lo = max(k0 - qq, 0)
                        sc2 = pscps.tile([128, 2 * QC], F32, tag="sc",
                                         name=f"sc{j}_{pn}_{kb}")
                        for hi, hr in ((0, 0), (1, 64)):
                            nc.tensor.matmul(
                                sc2[:, hi * QC + lo:(hi + 1) * QC],
                                lhsT=kT_t[j][hr:hr + 64, k0:k0 + 128],
                                rhs=qT_t[j][hr:hr + 64, qq + lo:qq + QC],
                                start=True, stop=True,
                            )
                        return sc2

                    def exp_pv(kb, sc2):
                        k0 = kb * 128
                        lo = max(k0 - q0, 0)
                        et = pexp.tile([128, 2 * QC], DT, tag="exp",
                                       name=f"et{j}_{p}_{kb}")
                        et3 = et[:].rearrange("p (h c) -> p h c", h=2)
                        sc3 = sc2[:].rearrange("p (h c) -> p h c", h=2)
                        nc.scalar.activation(
                            et3[:, :, lo:QC], sc3[:, :, lo:QC],
                            EXP, scale=1.0 / np.sqrt(HD),
                        )
                        if k0 >= q0:
                            nc.vector.tensor_tensor(
                                et3[:, :, lo:lo + 128],
                                et3[:, :, lo:lo + 128],
                                mask23, op=MULT,
                            )
                        for hi, vc in ((0, vcA), (1, vcB)):
                            nc.tensor.matmul(
                                at2[0:65, hi * QC + lo:(hi + 1) * QC],
                                lhsT=v_t[kb][:, vc:vc + HD + 1],
                                rhs=et[:, hi * QC + lo:(hi + 1) * QC],
                                start=(kb == 0), stop=(kb == nkb - 1),
                            )
                        if au is not None and k0 >= q0:
                            c = kb - 4 * p
                            nc.vector.tensor_copy(
                                au3[:, :, c * 128:(c + 1) * 128],
                                at3[:, :, c * 128:(c + 1) * 128],
                            )

                    if pend_next is not None:
                        pend = pend_next
                        pend_next = None
                    else:
                        pend = {kb: scores(kb) for kb in range(min(2, nkb))}
                    for kb in range(nkb):
                        want = min(len(pfill),
                                   ((kb + 1) * len(pfill))
                                   // max(nkb - 1, 2))
                        while fi < want:
                            pfill[fi]()
                            fi += 1
                        # exp_pv(kb) BEFORE scores(kb+2): the scps pool has
                        # 2 bufs, so scores(kb+2) reuses sc2(kb)'s buffer
                        # and its WAR dep must see exp(kb) already issued.
                        exp_pv(kb, pend.pop(kb))
                        fch = forced.get((p, kb))
                        if fch:
                            # close any open fill chunk first (aux psum
                            # groups must not interleave), then issue.
                            while fi not in bounds:
                                pfill[fi]()
                                fi += 1
                            for ch in fch:
                                for piece in ch:
                                    piece()
                        if kb == 2 and pending_norm:
                            # close any open fill chunk, then run the
                            # deferred normalize (gpsimd + DVE only), then
                            # append any late fill that reads the aT it
                            # writes (program order = data order in Tile).
                            while fi not in bounds:
                                pfill[fi]()
                                fi += 1
                            pending_norm.pop(0)()
                            for ch in late.get(p, []):
                                bounds.add(len(pfill))
                                pfill.extend(ch)
                            bounds.add(len(pfill))
                        if kb + 2 < nkb:
                            pend[kb + 2] = scores(kb + 2)
                    # cross-pass pipelining: issue the NEXT pass's first
                    # two score-pairs now, so the PE rolls straight from
                    # the last PV pair into them and the next pass's exp
                    # starts during this pass's denominator chain.  Issued
                    # only AFTER the last exp_pv: both psum buffers they
                    # rotate onto have had their exp issued (issuing one
                    # step earlier WAR-blocked the in-order PE behind a
                    # pending exp = the v11 +55us regression).
                    if p + 1 < NP:
                        qn = (p + 1) * QC
                        pend_next = {0: scores(0, qq=qn, pn=p + 1),
                                     1: scores(1, qq=qn, pn=p + 1)}
                    # drain fill, but hold back the final chunk: it is
                    # re-injected after the denominator chain below so the
                    # PE stream stays dense across the pass boundary while
                    # the DVE copy/reciprocal frees the at2 psum slot.
                    reserve = max((b for b in bounds if b < len(pfill)),
                                  default=len(pfill))
                    if reserve <= fi:
                        reserve = len(pfill)
                    while fi < reserve:
                        pfill[fi]()
                        fi += 1

                    if final and p == NP - 1:
                        # Very last pass: no next pass needs the at2 psum
                        # slot, so skip the au eviction copy and have the
                        # normalize read the psum directly, split into two
                        # q-halves so the epilogue can start the first
                        # out-projection s-blocks after only half a chain.
                        while fi < len(pfill):
                            pfill[fi]()
                            fi += 1

                        def norm_half(h, j=j, q0=q0, at2=at2, tn=f"{j}_{p}"):
                            at3 = at2[0:65, :].rearrange(
                                "p (hh q) -> p hh q", hh=2)
                            dnh = pdn.tile([1, 2, 256], F32, tag="dn",
                                           name=f"dnh{tn}_{h}")
                            nc.vector.tensor_copy(
                                dnh[:], at3[64:65, :, h * 256:(h + 1) * 256])
                            rch = pdn.tile([1, 2, 256], F32, tag="rc",
                                           name=f"rch{tn}_{h}")
                            nc.vector.reciprocal_approx_fast(rch[:], dnh[:])
                            bcbh = pbc.tile([64, 2, 256], F32, tag="bc",
                                            name=f"bcbh{tn}_{h}")
                            nc.gpsimd.partition_broadcast(bcbh[:], rch[:])
                            for hi, hr in ((0, 0), (1, 64)):
                                nc.vector.tensor_tensor(
                                    aT_t[j][hr:hr + 64,
                                            q0 + h * 256:q0 + (h + 1) * 256],
                                    at3[0:64, hi, h * 256:(h + 1) * 256],
                                    bcbh[:, hi, :],
                                    op=MULT,
                                )
                        pending_norm.append(lambda: norm_half(0))
                        pending_norm.append(lambda: norm_half(1))
                        continue

                    # one DVE copy frees the at2 psum slot; reciprocal of
                    # the denominator row runs now (DVE only); broadcast
                    # (gpsimd) + multiplies (DVE) are deferred one pass
                    # (see pending_norm).  No PE instruction in the chain:
                    # the old rank-1 PE broadcast was a 512-cycle stream
                    # with 1/128 of the array active -- dead PE time that
                    # also dragged the HAM activity monitor below its
                    # re-throttle threshold.
                    # au was fully evicted by the in-loop per-block copies;
                    # only the denominator extraction + reciprocal remain.
                    dn = pdn.tile([1, 2 * QC], F32, tag="dn", name=f"dn{j}_{p}")
                    nc.vector.tensor_copy(dn[:], au[64:65, :])
                    rc = pdn.tile([1, 2 * QC], F32, tag="rc", name=f"rc{j}_{p}")
                    nc.vector.reciprocal_approx_fast(rc[:], dn[:])
                    while fi < len(pfill):
                        pfill[fi]()
                        fi += 1

                    def norm_tail(j=j, q0=q0, au=au, rc=rc, tn=f"{j}_{p}"):
                        bcb = pbc.tile([64, 2 * QC], F32, tag="bc",
                                       name=f"bc{tn}")
                        nc.gpsimd.partition_broadcast(bcb[:], rc[0:1, :])
                        for hi, hr in ((0, 0), (1, 64)):
                            nc.vector.tensor_tensor(
                                aT_t[j][hr:hr + 64, q0:q0 + QC],
                                au[0:64, hi * QC:(hi + 1) * QC],
                                bcb[:, hi * QC:(hi + 1) * QC],
                                op=MULT,
                            )
                    pending_norm.append(norm_tail)

            # ---------------- schedule ----------------
            # prologue: only what pass 0 of window 0 needs up front --
            # q-chunk 0, k-chunk 0, V(0) -- interleaved with warm-up
            # matmuls: the prologue is DMA-paced (~100GB/s per queue), and
            # without filler the HAM monitor never sees 3.4us of sustained
            # activity, leaving the whole start of the kernel at 1.2GHz.
            qk0 = qk_pieces(0)          # [q0..q3, k0..k3]
            # one CONTIGUOUS burst long enough to cover a full free-running
            # 3.4us HAM window -- fragmented bursts never flip it to 2.4GHz
            warm_mms(18, pscps)
            for ci, ch in enumerate((qk0[0], qk0[4], v_pieces(0)[0])):
                for piece in ch:
                    piece()
                    warm_mms(2 if ci == 2 else 1, pscps)

            # Fill is spread so EVERY pass of every window carries some
            # full-array projection work: attention alone (64-row scores,
            # 65-col PV) sits at ~50% array activity, which drags the HAM
            # monitor below its threshold and re-throttles the PE to
            # 1.2GHz for whole passes.  QK(j+1) chunks [q0..q3, k0..k3]
            # are dealt one q + one k chunk per pass, with the sc3 pair
            # spilling into window j+1's early passes.
            vch = [v_pieces(sb)[0] for sb in range(1, NS)]   # V(1..15)
            qk1 = qk_pieces(1)
            forced = {
                (0, 0): [vch[0], qk0[1], qk0[5]],   # V1, q1, k1
                (0, 1): [vch[1]],                   # V2
                (0, 2): [vch[2]],                   # V3
                (1, 0): [qk0[2], qk0[6]],           # q2, k2
                (2, 0): [qk0[3], qk0[7]],           # q3, k3
            }
            fill = [vch[3:7] + [qk1[0]],
                    vch[7:11] + [qk1[4], qk1[1]],
                    vch[11:15] + [qk1[5], qk1[2]],
                    [qk1[6], qk1[3]]]
            attention_pair(0, fill, forced)

            # windows 1..2: attention(j) + QK(j+1) as fill (plus the sc3
            # leftovers of QK(j) in pass 0).
            qk_prev = qk1
            for j in range(1, NJ - 1):
                qk = qk_pieces(j + 1)
                fill = [[qk_prev[7], qk[0]], [qk[4], qk[1]],
                        [qk[5], qk[2]], [qk[6], qk[3]]]
                attention_pair(j, fill)
                qk_prev = qk

            # window 3: attention(3) + out-proj fill.  sb 4p..4p+3 needs
            # aT q-chunk p from ALL windows; window 3's normalize for
            # chunk p pops at pass p+1 kb==2, so sb 4p..4p+1 enter as
            # LATE fill right after that pop and the rest ride later
            # passes.  Passes 0-1 have little real fill available, so
            # warm-up chunks keep the HAM monitor from re-throttling.
            # Epilogue keeps only the final normalize + sb12..15.
            fill = [[qk_prev[7], warm_chunk(), warm_chunk(), warm_chunk()],
                    [warm_chunk(), warm_chunk(), warm_chunk()],
                    outproj_pieces(2) + outproj_pieces(3),
                    outproj_pieces(6) + outproj_pieces(7)]
            late = {
                1: outproj_pieces(0) + outproj_pieces(1),
                2: outproj_pieces(4) + outproj_pieces(5),
                3: outproj_pieces(8) + outproj_pieces(9),
            }
            attention_pair(NJ - 1, fill, late=late, final=True)

            # epilogue: sb10/11 moved OUT of pass 3's fill so their DVE
            # staging copies don't queue ahead of the final normalize
            # chain in the DVE FIFO; their matmuls (independent of the
            # final normalize) bridge the chain's latency on the PE and
            # keep the HAM monitor from re-throttling.  The final
            # normalize is split in two q-halves so sb12/13's
            # out-projection starts after only half a chain.
            warm_mms(4, pscps)
            pending_norm.pop(0)()       # final normalize, q-half 0
            for sb in (NS - 6, NS - 5):
                for ch in outproj_pieces(sb):
                    for piece in ch:
                        piece()
            pending_norm.pop(0)()       # final normalize, q-half 1
            for sb in (NS - 4, NS - 3, NS - 2, NS - 1):
                for ch in outproj_pieces(sb):
                    for piece in ch:
                        piece()

    nc.compile()
    return nc


def _get_nc(dt_mode):
    if dt_mode not in _CACHE:
        _CACHE[dt_mode] = _build(dt_mode)
    return _CACHE[dt_mode]


def make_mask2(np_dt):
    tri = np.triu(np.ones((128, 128), np.float32))
    return np.ascontiguousarray(np.concatenate([tri, tri], 1)).astype(np_dt)


def make_in_maps(x, Wq_w, Wq_b, Wk_w, Wk_b, Wv_w, Wv_b, Wo_w, Wo_b, np_dt):
    in_maps = []
    mask2 = make_mask2(np_dt)
    for core in range(N_CORES):
        b, half = core // 2, core % 2
        sl = slice(half * DH, (half + 1) * DH)
        xTb = np.ascontiguousarray(x[b].T)
        blocks = []
        for db in range(D // 128):
            rows = xTb[db * 128:(db + 1) * 128]
            blocks += [rows[:, 0:512], rows[:, 512:1024],
                       rows[:, 1024:2048].reshape(256, 512)]
        xt = np.ascontiguousarray(np.concatenate(blocks, 0))
        in_maps.append({
            "xT": xt.astype(np_dt),
            "wq": np.ascontiguousarray(Wq_w[:, sl]).astype(np_dt),
            "wk": np.ascontiguousarray(Wk_w[:, sl]).astype(np_dt),
            "wv": np.ascontiguousarray(Wv_w[:, sl]).astype(np_dt),
            "wo": np.ascontiguousarray(Wo_w[sl, :]).astype(np_dt),
            "bq": np.ascontiguousarray(Wq_b[sl].reshape(-1, 128).T),
            "bk": np.ascontiguousarray(Wk_b[sl].reshape(-1, 128).T),
            "mask2": mask2,
        })
    return in_maps


def kernel(x, Wq_w, Wq_b, Wk_w, Wk_b, Wv_w, Wv_b, Wo_w, Wo_b):
    from concourse.bass_utils import run_bass_kernel_spmd

    np_dt = ml_dtypes.bfloat16 if DT_MODE == "bf16" else np.float32

    args = [np.asarray(a, np.float32) for a in
            (x, Wq_w, Wq_b, Wk_w, Wk_b, Wv_w, Wv_b, Wo_w, Wo_b)]
    x, Wq_w, Wq_b, Wk_w, Wk_b, Wv_w, Wv_b, Wo_w, Wo_b = args

    nc = _get_nc(DT_MODE)
    in_maps = make_in_maps(x, Wq_w, Wq_b, Wk_w, Wk_b, Wv_w, Wv_b, Wo_w, Wo_b,
                           np_dt)
    res = run_bass_kernel_spmd(nc, in_maps, list(range(N_CORES)))

    # V-bias folded out of the kernel: (attn + bv) @ Wo = attn@Wo + bv@Wo,
    # a constant row added here together with Wo_b.
    crow = Wv_b @ Wo_w + Wo_b
    out = np.empty((B, S, D), np.float32)
    for b in range(B):
        out[b] = res.results[2 * b]["out"] + res.results[2 * b + 1]["out"] + crow
    return out

